# revision 19
# baseline (speedup 1.0000x reference)
"""Trainium2 Bass kernel: VQ codebook lookup + intra-sample attention +
cross-sample NxN attention, sharded over 8 NeuronCores.

The wall-clock of a call is dominated by host<->device transfer over the
axon tunnel (~15 ms/MB + per-array fixed cost), so the design minimizes
bytes and array count:

- ONE f32 input tensor per core [65, 928]: the core's own 800-row shard
  of x^T (with ones row), a 1/8 shard of the codebook distance matrix
  ct2 = [-2 C^T; ||c||^2], a 1/8 shard of the packed projection weights,
  and a tiny per-core mask selector. An on-device AllGather rebuilds the
  full x / ct2 / weights on every core (~2 MB over NeuronLink, ~free).
- ONE f16 output tensor per core [800, 130]: Z | X | argmin index. The
  exact f32 codebook gather happens on host from the indices.
- The block-diagonal same-sample mask of the cross-sample attention is
  folded into the score matmul: score rows get 64 extra contraction
  entries, a static sample-indicator one-hot on the q side (device
  generated via affine_select) and -1e30 * (own-sample one-hot) on the
  k side (expanded from the 2 KB per-core upload) -- so masked scores
  come out of the PE at -1e30 and exp() zeroes them for free.

Softmax is computed without max-subtraction (scores are O(5)); the
denominator comes from a ones-column appended to V in the same matmul.
"""

import numpy as np
from contextlib import ExitStack

import concourse.bass as bass
import concourse.tile as tile
from concourse import bacc, mybir
from concourse.bass_utils import run_bass_kernel_spmd
from concourse.masks import make_identity

F32 = mybir.dt.float32
F16 = mybir.dt.float16
BF16 = mybir.dt.bfloat16
AX = mybir.AxisListType
ALU = mybir.AluOpType
ACTF = mybir.ActivationFunctionType

BS, DN, SL, DIM = 64, 2, 50, 64
NE = 512                  # codebook size
N = BS * DN * SL          # 6400 flattened rows
NCORES = 8
PER = N // NCORES         # 800 rows per core
SAMP = DN * SL            # 100 rows per sample
NPAIR = PER // SL         # 16 (sample, domain) pairs per core
NSAMP = PER // SAMP       # 8 samples per core
NEG = -1.0e30

# packed input layout: [65, 928] f32 per core
#   cols 0:800    own shard of x^T (row 64 = ones)
#   cols 800:864  ct2 shard   (ct2 = [-2 C^T ; ||c||^2], cols c*64:(c+1)*64)
#   cols 864:913  weight shard (wpack [65, 392] padded, cols c*49:(c+1)*49)
#   cols 913:921  per-core mask selector mneg [8, 64] packed as [64, 8]
#                 (rows 0..63 only)
W_X, W_CT, W_WP = 800, 64, 49
C_CT = W_X                    # 800
C_WP = C_CT + W_CT            # 864
C_MN = C_WP + W_WP            # 913
W_IN = 928                    # 913 + 8 mneg + 7 pad; the
# AllGather payload size (65*W_IN*4 B) must be 64-byte aligned

# weight pack layout (columns of wpack [65, 392]):
OFF_QC, OFF_KC, OFF_VC = 0, 64, 128           # cs q, k, v(+ones col)
OFF_QI, OFF_KI, OFF_VI = 193, 257, 321        # is q, k, v(+ones col)
WPW = 392                                     # 386 + 6 pad

# fallback (no-collective) input layout: [65, 8112] f32 per core --
# own shard | full x^T | full ct2 | full wpack | mneg, all replicated
F_XA = 800
F_CT = F_XA + N               # 7200
F_WP = F_CT + NE              # 7712
F_MN = F_WP + WPW             # 8104
W_FB = F_MN + 8               # 8112

# output layout: [800, 130] f16 per core: z | x | idx | pad
OC_Z, OC_X, OC_I = 0, 64, 128
W_OUT = 130

TRACE = False
TRACE_KWARGS = {}
LAST_RESULTS = None
_CACHE = {}


def _ceil_div(a, b):
    return -(-a // b)


def _emit(ctx, tc, inf_d, out_d, use_ag=True):
    nc = tc.nc

    consts = ctx.enter_context(tc.tile_pool(name="consts", bufs=1))
    bigs = ctx.enter_context(tc.tile_pool(name="bigs", bufs=1))

    # own-shard loads (straight from this core's input)
    xown = consts.tile([DIM + 1, PER], F32, tag="xown")
    nc.sync.dma_start(out=xown, in_=inf_d[:, 0:W_X])
    mn_c = C_MN if use_ag else F_MN
    mneg = consts.tile([NSAMP, BS], F32, tag="mneg")
    nc.sync.dma_start(
        out=mneg.rearrange("t (a b) -> t a b", a=8),
        in_=inf_d[0:64, mn_c:mn_c + 8].rearrange("(t a) b -> t a b",
                                                 t=NSAMP))

    xall = consts.tile([DIM + 1, N], F32, tag="xall")
    ct2 = consts.tile([DIM + 1, NE], F32, tag="ct2")
    wf = consts.tile([DIM + 1, WPW], F32, tag="wf")
    if use_ag:
        # AllGather the packed input shards across the 8 cores, then
        # reassemble full x^T / ct2 / weights from the gathered blocks
        dram = ctx.enter_context(
            tc.tile_pool(name="dram", bufs=1, space="DRAM"))
        bounce = dram.tile([DIM + 1, W_IN], F32)
        gath = dram.tile([NCORES, DIM + 1, W_IN], F32, addr_space="Shared")
        nc.gpsimd.dma_start(bounce[:], inf_d)
        nc.gpsimd.collective_compute(
            "AllGather", ALU.bypass,
            replica_groups=[list(range(NCORES))],
            ins=[bounce.opt()], outs=[gath.opt()],
        )
        for c in range(NCORES):
            nc.sync.dma_start(out=xall[:, c * W_X:(c + 1) * W_X],
                              in_=gath[c, :, 0:C_CT])
            nc.sync.dma_start(out=ct2[:, c * W_CT:(c + 1) * W_CT],
                              in_=gath[c, :, C_CT:C_WP])
            nc.sync.dma_start(out=wf[:, c * W_WP:(c + 1) * W_WP],
                              in_=gath[c, :, C_WP:C_MN])
    else:
        # fallback: everything ships replicated in the (wider) input
        nc.sync.dma_start(out=xall[:, 0:N // 2],
                          in_=inf_d[:, F_XA:F_XA + N // 2])
        nc.sync.dma_start(out=xall[:, N // 2:N],
                          in_=inf_d[:, F_XA + N // 2:F_XA + N])
        nc.sync.dma_start(out=ct2, in_=inf_d[:, F_CT:F_CT + NE])
        nc.sync.dma_start(out=wf, in_=inf_d[:, F_WP:F_WP + WPW])

    ident = consts.tile([128, 128], F32, tag="ident")
    make_identity(nc, ident)

    # ---- bf16 casts ----
    xbf = consts.tile([DIM + 1, N], BF16, tag="xbf")
    nc.any.tensor_copy(xbf[:, 0:N // 2], xall[:, 0:N // 2])
    nc.any.tensor_copy(xbf[:, N // 2:N], xall[:, N // 2:N])
    xob = consts.tile([DIM + 1, PER], BF16, tag="xob")
    nc.any.tensor_copy(xob, xown)
    wbf = consts.tile([DIM + 1, WPW], BF16, tag="wbf")
    nc.any.tensor_copy(wbf, wf)
    mnegb = consts.tile([NSAMP, BS], BF16, tag="mnegb")
    nc.any.tensor_copy(mnegb, mneg)

    # ---- persistent SBUF intermediates ----
    # mask-bias factors live in separate partition-0 tiles; the score
    # matmul accumulates proj.proj + ind.kaug (two K=64 products), so no
    # op ever has to move data across partitions.
    qcT = bigs.tile([DIM, N], BF16, tag="qcT")
    kcT = bigs.tile([DIM, PER], BF16, tag="kcT")
    ind = bigs.tile([DIM, N], BF16, tag="ind")
    kaug = bigs.tile([DIM, PER], BF16, tag="kaug")
    qiT = bigs.tile([DIM, PER], BF16, tag="qiT")
    kiT = bigs.tile([DIM, PER], BF16, tag="kiT")
    vcaug = bigs.tile([128, 50 * 65], BF16, tag="vcaug")
    viaug = bigs.tile([SL, NPAIR * 65], BF16, tag="viaug")

    # static sample-indicator rows: ind[s, j] = 1 iff j//100 == s
    nc.gpsimd.memset(ind, 1.0)
    nc.gpsimd.affine_select(
        out=ind, in_=ind, compare_op=ALU.is_ge,
        fill=0.0, base=0, channel_multiplier=-SAMP, pattern=[[1, N]])
    nc.gpsimd.affine_select(
        out=ind, in_=ind, compare_op=ALU.is_ge,
        fill=0.0, base=SAMP - 1, channel_multiplier=SAMP, pattern=[[-1, N]])

    # E[t, i] = 1 iff i//100 == t  (t = own-sample slot 0..7)
    E = consts.tile([NSAMP, PER], BF16, tag="E")
    nc.gpsimd.memset(E, 1.0)
    nc.gpsimd.affine_select(
        out=E, in_=E, compare_op=ALU.is_ge,
        fill=0.0, base=0, channel_multiplier=-SAMP, pattern=[[1, PER]])
    nc.gpsimd.affine_select(
        out=E, in_=E, compare_op=ALU.is_ge,
        fill=0.0, base=SAMP - 1, channel_multiplier=SAMP, pattern=[[-1, PER]])

    # iota row values 0..511 for argmin-index extraction
    iotav = consts.tile([128, NE], F32, tag="iotav")
    nc.gpsimd.iota(iotav, pattern=[[1, NE]], base=0, channel_multiplier=0,
                   allow_small_or_imprecise_dtypes=True)

    # f16 output staging
    zst = consts.tile([SL, NPAIR, DIM], F16, tag="zst")
    idxs = consts.tile([SAMP, NSAMP], F16, tag="idxs")

    # ================= prep phase (rotating psum slots) =================
    # every prep psum tile is <= 1 bank (2 KB/partition); one tag, 4 slots
    _n_small = [0]

    with tc.tile_pool(name="prep", bufs=4, space="PSUM") as prep:

        def small_tile(shape):
            _n_small[0] += 1
            return prep.tile(shape, F32, tag="small",
                             name=f"small{_n_small[0]}")

        # mask-bias factor: kaug = mneg^T @ E  ([64, 800], -1e30 one-hots)
        for o, wd in ((0, 512), (512, PER - 512)):
            ka = small_tile([DIM, 512])
            nc.tensor.matmul(ka[:, 0:wd], mnegb, E[:, o:o + wd],
                             start=True, stop=True)
            nc.any.tensor_copy(kaug[:, o:o + wd], ka[:, 0:wd])

        # cs q projection over all 6400 rows
        PJ = 512
        for k in range(_ceil_div(N, PJ)):
            o = k * PJ
            wd = min(PJ, N - o)
            ps = small_tile([DIM, PJ])
            nc.tensor.matmul(ps[:, 0:wd], wbf[:, OFF_QC:OFF_QC + 64],
                             xbf[:, o:o + wd], start=True, stop=True)
            nc.any.tensor_copy(qcT[0:64, o:o + wd], ps[:, 0:wd])

        # own-row projections: cs k, is q, is k
        for dst, off in ((kcT, OFF_KC), (qiT, OFF_QI), (kiT, OFF_KI)):
            for o, wd in ((0, 512), (512, PER - 512)):
                pj = small_tile([DIM, 512])
                nc.tensor.matmul(pj[:, 0:wd], wbf[:, off:off + 64],
                                 xob[:, o:o + wd], start=True, stop=True)
                nc.any.tensor_copy(dst[0:DIM, o:o + wd], pj[:, 0:wd])

        # cs V rows (+bias +ones col) for ALL rows, groups of 7 jb
        for g in range(_ceil_div(50, 7)):
            nj = min(7, 50 - g * 7)
            vt = small_tile([128, 7, 65])
            for j in range(nj):
                jb = g * 7 + j
                nc.tensor.matmul(vt[:, j, :], xbf[:, jb * 128:(jb + 1) * 128],
                                 wbf[:, OFF_VC:OFF_VC + 65],
                                 start=True, stop=True)
            nc.any.tensor_copy(
                vcaug[:, g * 7 * 65:(g * 7 + nj) * 65], vt[:, 0:nj, :])

        # is V rows (+bias +ones col) for own 16 pairs
        for g in range(_ceil_div(NPAIR, 7)):
            nj = min(7, NPAIR - g * 7)
            vp = small_tile([SL, 7, 65])
            for j in range(nj):
                p = g * 7 + j
                nc.tensor.matmul(vp[:, j, :], xob[:, p * SL:(p + 1) * SL],
                                 wbf[:, OFF_VI:OFF_VI + 65],
                                 start=True, stop=True)
            nc.any.tensor_copy(
                viaug[:, g * 7 * 65:(g * 7 + nj) * 65], vp[:, 0:nj, :])

        # ---- VQ: argmin_c(-2 x.c + ||c||^2) over own rows, idx out ----
        vqs = ctx.enter_context(tc.tile_pool(name="vqs", bufs=2))
        for k in range(NSAMP):
            co = k * SAMP
            dps = small_tile([SAMP, NE])
            nc.tensor.matmul(dps, xown[:, co:co + SAMP], ct2,
                             start=True, stop=True)
            minv = vqs.tile([SAMP, 1], F32, tag="minv", name=f"minv{k}")
            nc.vector.tensor_reduce(out=minv, in_=dps, axis=AX.X, op=ALU.min)
            oh = vqs.tile([SAMP, NE], F32, tag="oh", name=f"oh{k}")
            nc.vector.tensor_scalar(out=oh, in0=dps, scalar1=minv,
                                    scalar2=None, op0=ALU.is_equal)
            nc.vector.tensor_mul(oh, oh, iotav[0:SAMP, :])
            idxc = vqs.tile([SAMP, 1], F32, tag="idxc", name=f"idxc{k}")
            nc.vector.tensor_reduce(out=idxc, in_=oh, axis=AX.X, op=ALU.add)
            nc.any.tensor_copy(idxs[:, k:k + 1], idxc)
        nc.sync.dma_start(
            out=out_d[:, OC_I:OC_I + 1].rearrange("(k p) e -> p (k e)",
                                                  p=SAMP),
            in_=idxs)

        # ---- intra-sample attention: 16 independent 50x50 ----
        iss = ctx.enter_context(tc.tile_pool(name="iss", bufs=2))
        isb = ctx.enter_context(tc.tile_pool(name="isb", bufs=1))
        est_is = isb.tile([SL, NPAIR * SL], BF16, tag="est_is")
        for h in range(2):
            stt = small_tile([SL, 8, DIM])
            for j in range(8):
                p = h * 8 + j
                nc.tensor.matmul(stt[:, j, 0:SL], qiT[:, p * SL:(p + 1) * SL],
                                 kiT[:, p * SL:(p + 1) * SL],
                                 start=True, stop=True)
            nc.scalar.activation(est_is[:, h * 8 * SL:(h + 1) * 8 * SL],
                                 stt[:, :, 0:SL], ACTF.Exp)
        for g in range(_ceil_div(NPAIR, 7)):
            nj = min(7, NPAIR - g * 7)
            zz = small_tile([SL, 7, 65])
            for j in range(nj):
                p = g * 7 + j
                nc.tensor.matmul(zz[:, j, :], est_is[:, p * SL:(p + 1) * SL],
                                 viaug[:, p * 65:p * 65 + 65],
                                 start=True, stop=True)
            drz = iss.tile([SL, 7], F32, tag="drz", name=f"drz{g}")
            nc.vector.reciprocal(drz[:, 0:nj], zz[:, 0:nj, 64])
            for j in range(nj):
                p = g * 7 + j
                nc.vector.tensor_scalar_mul(zst[:, p, :], zz[:, j, 0:DIM],
                                            drz[:, j:j + 1])
        nc.sync.dma_start(
            out=out_d[:, OC_Z:OC_Z + DIM].rearrange("(q t) e -> t q e", t=SL),
            in_=zst)

    # ================= cross-sample attention =================
    # PSUM budget (8 banks): st 2x2 + ut 1x2 + ep 2x1 = 8
    csp = ctx.enter_context(tc.tile_pool(name="csp", bufs=2, space="PSUM"))
    utp = ctx.enter_context(tc.tile_pool(name="utp", bufs=1, space="PSUM"))
    epp = ctx.enter_context(tc.tile_pool(name="epp", bufs=2, space="PSUM"))
    css = ctx.enter_context(tc.tile_pool(name="css", bufs=2))
    cse = ctx.enter_context(tc.tile_pool(name="cse", bufs=2))

    ut = utp.tile([65, PER], F32, tag="ut")  # [aug_e, own_i] accumulator
    for jb in range(50):
        jsl = slice(jb * 128, (jb + 1) * 128)
        st = csp.tile([128, PER], F32, tag="st")
        nc.tensor.matmul(st[:, 0:512], qcT[:, jsl], kcT[:, 0:512],
                         start=True, stop=False)
        nc.tensor.matmul(st[:, 0:512], ind[:, jsl], kaug[:, 0:512],
                         start=False, stop=True)
        nc.tensor.matmul(st[:, 512:PER], qcT[:, jsl], kcT[:, 512:PER],
                         start=True, stop=False)
        nc.tensor.matmul(st[:, 512:PER], ind[:, jsl], kaug[:, 512:PER],
                         start=False, stop=True)
        est = css.tile([128, PER], BF16, tag="est")
        nc.scalar.activation(est, st, ACTF.Exp)
        nc.tensor.matmul(ut[:, 0:512], vcaug[:, jb * 65:jb * 65 + 65],
                         est[:, 0:512], start=(jb == 0), stop=(jb == 49),
                         skip_group_check=True)
        nc.tensor.matmul(ut[:, 512:PER], vcaug[:, jb * 65:jb * 65 + 65],
                         est[:, 512:PER], start=(jb == 0), stop=(jb == 49),
                         skip_group_check=True)

    ut_s = cse.tile([65, PER], F32, tag="ut_s")
    nc.vector.tensor_copy(ut_s, ut)
    for g in range(2):
        xp = epp.tile([SAMP, 4, 65], F32, tag="ep", name=f"xp{g}")
        for k in range(4):
            s = g * 4 + k
            nc.tensor.transpose(xp[:, k, :], ut_s[:, s * SAMP:(s + 1) * SAMP],
                                ident[0:65, 0:65])
        dr = cse.tile([SAMP, 4], F32, tag="dr")
        nc.vector.reciprocal(dr, xp[:, :, 64])
        xg = cse.tile([SAMP, 4, DIM], F16, tag="xg")
        for k in range(4):
            nc.vector.tensor_scalar_mul(xg[:, k, :], xp[:, k, 0:DIM],
                                        dr[:, k:k + 1])
        nc.sync.dma_start(
            out=out_d[g * 400:(g + 1) * 400, OC_X:OC_X + DIM].rearrange(
                "(s p) e -> p s e", p=SAMP),
            in_=xg)


def _build(use_ag=True):
    nc = bacc.Bacc("TRN2", target_bir_lowering=False, debug=False,
                   num_devices=NCORES)
    inf_d = nc.dram_tensor("inf", [DIM + 1, W_IN if use_ag else W_FB], F32,
                           kind="ExternalInput").ap()
    out_d = nc.dram_tensor("out", [PER, W_OUT], F16,
                           kind="ExternalOutput").ap()

    with tile.TileContext(nc) as tc:
        with ExitStack() as ctx:
            _emit(ctx, tc, inf_d, out_d, use_ag=use_ag)
    nc.compile()
    return nc


def _host_inputs(x, code_book,
                 Wq_is, bq_is, Wk_is, bk_is, Wv_is, bv_is,
                 Wq_cs, bq_cs, Wk_cs, bk_cs, Wv_cs, bv_cs,
                 use_ag=True):
    f = np.float32
    xaug = np.empty((DIM + 1, N), f)
    xaug[0:DIM] = np.asarray(x, f).reshape(N, DIM).T
    xaug[DIM] = 1.0

    def waug(W, b):                                   # [65, 64]
        return np.concatenate(
            [np.asarray(W, f), np.asarray(b, f).reshape(1, DIM)], axis=0)

    def waug_ones(W, b):                              # [65, 65]
        out = np.zeros((DIM + 1, DIM + 1), f)
        out[:DIM, :DIM] = np.asarray(W, f)
        out[DIM, :DIM] = np.asarray(b, f)
        out[DIM, DIM] = 1.0
        return out

    wpack = np.zeros((DIM + 1, WPW), f)
    wpack[:, OFF_QC:OFF_QC + 64] = waug(Wq_cs, bq_cs)
    wpack[:, OFF_KC:OFF_KC + 64] = waug(Wk_cs, bk_cs)
    wpack[:, OFF_VC:OFF_VC + 65] = waug_ones(Wv_cs, bv_cs)
    wpack[:, OFF_QI:OFF_QI + 64] = waug(Wq_is, bq_is)
    wpack[:, OFF_KI:OFF_KI + 64] = waug(Wk_is, bk_is)
    wpack[:, OFF_VI:OFF_VI + 65] = waug_ones(Wv_is, bv_is)

    C = np.asarray(code_book, f)
    ct2 = np.empty((DIM + 1, NE), f)
    ct2[0:DIM] = -2.0 * C.T
    ct2[DIM] = (C * C).sum(axis=1)

    inf_all = np.zeros((NCORES, DIM + 1, W_IN if use_ag else W_FB), f)
    inf_all[:, :, 0:W_X] = xaug.reshape(DIM + 1, NCORES, W_X).transpose(
        1, 0, 2)
    if use_ag:
        inf_all[:, :, C_CT:C_WP] = ct2.reshape(
            DIM + 1, NCORES, W_CT).transpose(1, 0, 2)
        inf_all[:, :, C_WP:C_MN] = wpack.reshape(
            DIM + 1, NCORES, W_WP).transpose(1, 0, 2)
        mn_c = C_MN
    else:
        inf_all[:, :, F_XA:F_CT] = xaug
        inf_all[:, :, F_CT:F_WP] = ct2
        inf_all[:, :, F_WP:F_MN] = wpack
        mn_c = F_MN
    for c in range(NCORES):
        mneg = np.zeros((NSAMP, BS), f)
        for t in range(NSAMP):
            mneg[t, NSAMP * c + t] = NEG
        inf_all[c, 0:64, mn_c:mn_c + 8] = mneg.reshape(64, 8)

    return [{"inf": np.ascontiguousarray(inf_all[c])} for c in range(NCORES)]


USE_AG = True       # primary path; auto-falls-back if collectives fail


def _run_mode(inputs, use_ag):
    key = "nc_ag" if use_ag else "nc_fb"
    if key not in _CACHE:
        _CACHE[key] = _build(use_ag=use_ag)
    in_maps = _host_inputs(**inputs, use_ag=use_ag)
    return run_bass_kernel_spmd(_CACHE[key], in_maps, list(range(NCORES)),
                                trace=TRACE, trace_kwargs=TRACE_KWARGS)


def kernel(**inputs):
    global LAST_RESULTS, USE_AG
    if USE_AG:
        try:
            res = _run_mode(inputs, True)
        except Exception:
            # AllGather unsupported on this terminal (or transient failure):
            # retry once, then switch to the replicated-input fallback
            try:
                res = _run_mode(inputs, True)
            except Exception:
                USE_AG = False
                res = _run_mode(inputs, False)
    else:
        res = _run_mode(inputs, False)
    LAST_RESULTS = res
    out = np.concatenate([res.results[c]["out"] for c in range(NCORES)],
                         axis=0)                       # [6400, 130] f16
    shape = (BS, DN, SL, DIM)
    z = out[:, OC_Z:OC_Z + DIM].astype(np.float32).reshape(shape)
    xx = out[:, OC_X:OC_X + DIM].astype(np.float32).reshape(shape)
    idx = np.clip(out[:, OC_I].astype(np.int64), 0, NE - 1)
    cb = np.asarray(inputs["code_book"], np.float32)
    quant = cb[idx].reshape(shape)
    return quant, z, xx
